# revision 17
# baseline (speedup 1.0000x reference)
"""Trainium2 Bass kernel for NeuromodulatedAttentionWithElectrodes.

Math simplification (verified ~1e-6 vs the jax reference): the dopamine/
serotonin "mod" term is a per-row constant added to the score matrix;
row-std (any ddof) is shift-invariant and softmax is shift-invariant, so
that whole pipeline cancels. Per batch b:

    Qe, Ke, Ve = Q[b]+e, K[b]+e, V[b]+e        (e = emb[electrode_ids])
    Sraw = Qe @ Ke^T                           [N, N]
    r[n] = 1/rowstd(Sraw, ddof=1)
    P    = exp(Sraw * r[:, None]);  rs = rowsum(P)
    AO   = (P @ Ve) / rs[:, None]              [N, D]
    out_b = AO.flat @ fc_w.T + fc_b            [C]

Device computes Sraw/P/AO; the tiny fc contraction (0.025% of FLOPs) and
input prep (e-add, transpose, quantize) run on host.

Scores use fp8(e4m3) DoubleRow matmuls with one-sided error compensation:
    Sraw ~= (Q8 + dQ8) @ K8^T,  Q8 = fp8(Qe), dQ8 = fp8(Qe - Q8)
(end-to-end rel err ~1e-2 vs 2e-2 budget; AO matmul stays bf16).

Sharding: data-parallel over batch, 8 batches per core on 8 cores.
"""

import numpy as np

B, N, D, C = 64, 512, 1024, 4
NCORES = 8
BPC = B // NCORES  # batches per core
P = 128            # partitions
NB = N // P        # 4 row blocks
DB = D // P        # 8 d chunks
DP = DB // 2       # 4 DoubleRow chunk-pairs

_prog_cache = {}
_last_in_maps = None
COMP = False  # one-sided fp8 error compensation for the score matmul


def _build_program():
    from contextlib import ExitStack

    import concourse.mybir as mybir
    import concourse.tile as tile
    from concourse import bacc
    from concourse.bass import ts
    from concourse.masks import make_identity

    f32 = mybir.dt.float32
    bf16 = mybir.dt.bfloat16
    fp8 = mybir.dt.float8e4
    AF = mybir.ActivationFunctionType
    ALU = mybir.AluOpType
    DR = mybir.MatmulPerfMode.DoubleRow

    # Keep Exp+Ln resident in one ACT table set (avoid reload ping-pong).
    from concourse import hw_specs as _hws
    if not getattr(bacc, "_act_tbl_patched", False):
        _orig_gat = _hws.get_activation_tables

        def _patched_gat(arch):
            t = _orig_gat(arch)
            AF_ = mybir.ActivationFunctionType
            for name, fns in t.items():
                if name != "natural_log_exp_and_others":
                    fns.discard(AF_.Exp)
                    fns.discard(AF_.Ln)
            return t

        bacc.get_activation_tables = _patched_gat
        bacc._act_tbl_patched = True

    nc = bacc.Bacc("TRN2", target_bir_lowering=False, debug=False,
                   num_devices=NCORES)
    q8_d = nc.declare_dram_parameter("q8t", [BPC, D, N], fp8, isOutput=False)
    dq8_d = (nc.declare_dram_parameter("dq8t", [BPC, D, N], fp8,
                                       isOutput=False) if COMP else None)
    k8_d = nc.declare_dram_parameter("k8t", [BPC, D, N], fp8, isOutput=False)
    ve_d = nc.declare_dram_parameter("veb", [BPC, N, D], bf16, isOutput=False)
    ao_d = nc.declare_dram_parameter("ao", [BPC, N, D], bf16, isOutput=True)

    with tile.TileContext(nc) as tc, ExitStack() as ctx:
        const_p = ctx.enter_context(tc.tile_pool(name="const", bufs=1))
        in_p = ctx.enter_context(tc.tile_pool(name="inp", bufs=3))
        p_p = ctx.enter_context(tc.tile_pool(name="p", bufs=8))
        trp_p = ctx.enter_context(tc.tile_pool(name="trp", bufs=8))
        ao_p = ctx.enter_context(tc.tile_pool(name="ao", bufs=3))
        st_p = ctx.enter_context(tc.tile_pool(name="st", bufs=24))
        psS = ctx.enter_context(tc.tile_pool(name="psS", bufs=2, space="PSUM"))
        psA = ctx.enter_context(tc.tile_pool(name="psA", bufs=3, space="PSUM"))
        psO = ctx.enter_context(tc.tile_pool(name="psO", bufs=3, space="PSUM"))

        ident = const_p.tile([P, P], bf16, tag="ident")
        make_identity(nc, ident[:, :])

        # Software pipeline with 1-batch skew: segment b emits loads+scores+
        # exp for batch b, then PT+AO+store for batch b-1 (whose exp chain
        # finished during segment b's scores) so PE never stalls on the
        # stats->exp latency.
        stage = {}   # b -> (p_sb, inv, ve)
        loaded = {}  # b -> (q8, k8, dq8, ve)

        def emit_loads(b):
            q8 = in_p.tile([P, DB * N], fp8, tag="q8")
            k8 = in_p.tile([P, DB * N], fp8, tag="k8")
            srcs = [(k8, k8_d), (q8, q8_d)]
            dq8 = None
            if COMP:
                dq8 = in_p.tile([P, DB * N], fp8, tag="dq8")
                srcs.append((dq8, dq8_d))
            # split loads by chunk halves so the first score matmuls can
            # start before the whole tensor lands
            h = DB // 2
            views = []
            for sb, src in srcs:
                sbv = sb[:, :].rearrange("p (c n) -> p c n", c=DB)
                srcv = src[b].rearrange("(c p) n -> p c n", p=P)
                views.append((sbv, srcv))
            for lo, hi in ((0, h), (h, DB)):
                for sbv, srcv in views:
                    nc.sync.dma_start(sbv[:, lo:hi, :], srcv[:, lo:hi, :])
            ve = in_p.tile([P, NB * D], bf16, tag="ve")
            nc.sync.dma_start(
                ve[:, :].rearrange("p (t d) -> p t d", t=NB),
                ve_d[b].rearrange("(t p) d -> p t d", p=P))
            loaded[b] = (q8, k8, dq8, ve)

        def emit_front(b):
            q8, k8, dq8, ve = loaded.pop(b)
            q8v = q8[:, :].rearrange("p (c n) -> p c n", c=DB)
            k8v = k8[:, :].rearrange("p (c n) -> p c n", c=DB)
            groups = [q8v]
            if COMP:
                groups.append(dq8[:, :].rearrange("p (c n) -> p c n", c=DB))

            p_sb, inv = [], []
            for t in range(NB):
                s_ps = psS.tile([P, N], f32, tag="s")
                for cp in range(DP):
                    for g, qv in enumerate(groups):
                        nc.tensor.matmul(
                            s_ps[:, :],
                            qv[:, 2 * cp:2 * cp + 2, ts(t, P)],
                            k8v[:, 2 * cp:2 * cp + 2, :],
                            start=(cp == 0 and g == 0),
                            stop=(cp == DP - 1 and g == len(groups) - 1),
                            perf_mode=DR)
                st6 = st_p.tile([P, 6], f32, tag="st6")
                nc.vector.bn_stats(st6[:, :], s_ps[:, :])
                mv = st_p.tile([P, 2], f32, tag="mv")
                nc.vector.bn_aggr(mv[:, :], st6[:, :])
                # r = 1/sd = exp(-0.5*ln(var*N/(N-1)))
                lnv = st_p.tile([P, 1], f32, tag="lnv")
                nc.scalar.activation(lnv[:, :], mv[:, 1:2], AF.Ln,
                                     scale=float(N) / (N - 1.0))
                r = st_p.tile([P, 1], f32, tag="r")
                nc.scalar.activation(r[:, :], lnv[:, :], AF.Exp, scale=-0.5)
                pt_ = p_p.tile([P, N], bf16, tag="p")
                rs = st_p.tile([P, 1], f32, tag="rs")
                nc.scalar.activation(pt_[:, :], s_ps[:, :], AF.Exp,
                                     scale=r[:, :], accum_out=rs[:, :])
                iv = st_p.tile([P, 1], f32, tag="iv")
                nc.vector.reciprocal(iv[:, :], rs[:, :])
                p_sb.append(pt_)
                inv.append(iv)
            stage[b] = (p_sb, inv, ve)

        def emit_back(b):
            p_sb, inv, ve = stage.pop(b)
            ao_sb = ao_p.tile([P, NB * D], bf16, tag="ao")
            xts = []
            for t in range(NB):
                # transpose P_t -> [m, n-block-t], depends only on exp(t)
                tp = psA.tile([P, N], bf16, tag="tpose")
                for mc in range(NB):
                    nc.tensor.matmul(
                        tp[:, ts(mc, P)], p_sb[t][:, ts(mc, P)],
                        ident[:, :], is_transpose=True,
                        start=True, stop=True)
                xt = trp_p.tile([P, N], bf16, tag="ptr")
                if t % 2 == 0:
                    nc.scalar.copy(xt[:, :], tp[:, :])
                else:
                    nc.vector.tensor_copy(xt[:, :], tp[:, :])
                xts.append(xt)
            for t in range(NB):
                xt = xts[t]
                for dt_ in range(2):
                    a_ps = psO.tile([P, 512], f32, tag="aops")
                    for mc in range(NB):
                        nc.tensor.matmul(
                            a_ps[:, :], xt[:, ts(mc, P)],
                            ve[:, mc * D + dt_ * 512: mc * D + dt_ * 512 + 512],
                            start=(mc == 0), stop=(mc == NB - 1))
                    dst = ao_sb[:, t * D + dt_ * 512: t * D + dt_ * 512 + 512]
                    if dt_ == 0:
                        nc.scalar.mul(dst, a_ps[:, :], inv[t][:, :])
                    else:
                        nc.vector.tensor_scalar_mul(dst, a_ps[:, :],
                                                    inv[t][:, :])
                nc.sync.dma_start(
                    ao_d[b, t * P:(t + 1) * P, :],
                    ao_sb[:, ts(t, D)])

        emit_loads(0)
        for b in range(BPC + 1):
            if b < BPC:
                if b + 1 < BPC:
                    emit_loads(b + 1)
                emit_front(b)
            if b > 0:
                emit_back(b - 1)

    nc.compile()
    return nc


def kernel(**inputs):
    import ml_dtypes
    from concourse.bass_utils import run_bass_kernel_spmd

    global _last_in_maps
    f8 = ml_dtypes.float8_e4m3
    bf = ml_dtypes.bfloat16

    Q = np.asarray(inputs["Q"], dtype=np.float32)
    K = np.asarray(inputs["K"], dtype=np.float32)
    V = np.asarray(inputs["V"], dtype=np.float32)
    ids = np.asarray(inputs["electrode_ids"]).astype(np.int64)
    emb = np.asarray(inputs["emb"], dtype=np.float32)
    fc_w = np.asarray(inputs["fc_w"], dtype=np.float32)
    fc_b = np.asarray(inputs["fc_b"], dtype=np.float32)

    e = emb[ids]                                  # [N, D]
    QT = np.ascontiguousarray((Q + e).transpose(0, 2, 1))   # [B, D, N]
    KT = np.ascontiguousarray((K + e).transpose(0, 2, 1))
    Q8 = QT.astype(f8)
    K8 = KT.astype(f8)
    dQ8 = (QT - Q8.astype(np.float32)).astype(f8) if COMP else None
    Veb = np.ascontiguousarray((V + e).astype(bf))          # [B, N, D]

    if "prog" not in _prog_cache:
        _prog_cache["prog"] = _build_program()
    nc = _prog_cache["prog"]

    in_maps = []
    for i in range(NCORES):
        sl = slice(i * BPC, (i + 1) * BPC)
        m = {
            "q8t": np.ascontiguousarray(Q8[sl]),
            "k8t": np.ascontiguousarray(K8[sl]),
            "veb": Veb[sl],
        }
        if COMP:
            m["dq8t"] = np.ascontiguousarray(dQ8[sl])
        in_maps.append(m)
    _last_in_maps = in_maps
    res = run_bass_kernel_spmd(nc, in_maps, list(range(NCORES)))
    AO = np.concatenate([np.asarray(r["ao"]) for r in res.results], axis=0)
    AO = AO.astype(np.float32).reshape(B, N * D)
    out = AO @ fc_w.T + fc_b
    return np.ascontiguousarray(out.astype(np.float32))


# revision 18
# speedup vs baseline: 1.0251x; 1.0251x over previous
"""Trainium2 Bass kernel for NeuromodulatedAttentionWithElectrodes.

Math simplification (verified ~1e-6 vs the jax reference): the dopamine/
serotonin "mod" term is a per-row constant added to the score matrix;
row-std (any ddof) is shift-invariant and softmax is shift-invariant, so
that whole pipeline cancels. Per batch b:

    Qe, Ke, Ve = Q[b]+e, K[b]+e, V[b]+e        (e = emb[electrode_ids])
    Sraw = Qe @ Ke^T                           [N, N]
    r[n] = 1/rowstd(Sraw, ddof=1)
    P    = exp(Sraw * r[:, None]);  rs = rowsum(P)
    AO   = (P @ Ve) / rs[:, None]              [N, D]
    out_b = AO.flat @ fc_w.T + fc_b            [C]

Device computes Sraw/P/AO; the tiny fc contraction (0.025% of FLOPs) and
input prep (e-add, transpose, quantize) run on host.

Scores use fp8(e4m3) DoubleRow matmuls with one-sided error compensation:
    Sraw ~= (Q8 + dQ8) @ K8^T,  Q8 = fp8(Qe), dQ8 = fp8(Qe - Q8)
(end-to-end rel err ~1e-2 vs 2e-2 budget; AO matmul stays bf16).

Sharding: data-parallel over batch, 8 batches per core on 8 cores.
"""

import numpy as np

B, N, D, C = 64, 512, 1024, 4
NCORES = 8
BPC = B // NCORES  # batches per core
P = 128            # partitions
NB = N // P        # 4 row blocks
DB = D // P        # 8 d chunks
DP = DB // 2       # 4 DoubleRow chunk-pairs

_prog_cache = {}
_last_in_maps = None
COMP = False  # one-sided fp8 error compensation for the score matmul


def _build_program():
    from contextlib import ExitStack

    import concourse.mybir as mybir
    import concourse.tile as tile
    from concourse import bacc
    from concourse.bass import ts
    from concourse.masks import make_identity

    f32 = mybir.dt.float32
    bf16 = mybir.dt.bfloat16
    fp8 = mybir.dt.float8e4
    AF = mybir.ActivationFunctionType
    ALU = mybir.AluOpType
    DR = mybir.MatmulPerfMode.DoubleRow

    # Keep Exp+Ln resident in one ACT table set (avoid reload ping-pong).
    from concourse import hw_specs as _hws
    if not getattr(bacc, "_act_tbl_patched", False):
        _orig_gat = _hws.get_activation_tables

        def _patched_gat(arch):
            t = _orig_gat(arch)
            AF_ = mybir.ActivationFunctionType
            for name, fns in t.items():
                if name != "natural_log_exp_and_others":
                    fns.discard(AF_.Exp)
                    fns.discard(AF_.Ln)
            return t

        bacc.get_activation_tables = _patched_gat
        bacc._act_tbl_patched = True

    nc = bacc.Bacc("TRN2", target_bir_lowering=False, debug=False,
                   num_devices=NCORES)
    q8_d = nc.declare_dram_parameter("q8t", [BPC, D, N], fp8, isOutput=False)
    dq8_d = (nc.declare_dram_parameter("dq8t", [BPC, D, N], fp8,
                                       isOutput=False) if COMP else None)
    k8_d = nc.declare_dram_parameter("k8t", [BPC, D, N], fp8, isOutput=False)
    ve_d = nc.declare_dram_parameter("veb", [BPC, N, D], bf16, isOutput=False)
    ao_d = nc.declare_dram_parameter("ao", [BPC, N, D], bf16, isOutput=True)

    with tile.TileContext(nc) as tc, ExitStack() as ctx:
        const_p = ctx.enter_context(tc.tile_pool(name="const", bufs=1))
        in_p = ctx.enter_context(tc.tile_pool(name="inp", bufs=3))
        p_p = ctx.enter_context(tc.tile_pool(name="p", bufs=8))
        trp_p = ctx.enter_context(tc.tile_pool(name="trp", bufs=8))
        ao_p = ctx.enter_context(tc.tile_pool(name="ao", bufs=3))
        st_p = ctx.enter_context(tc.tile_pool(name="st", bufs=24))
        psS = ctx.enter_context(tc.tile_pool(name="psS", bufs=2, space="PSUM"))
        psA = ctx.enter_context(tc.tile_pool(name="psA", bufs=4, space="PSUM"))
        psO = ctx.enter_context(tc.tile_pool(name="psO", bufs=2, space="PSUM"))

        ident = const_p.tile([P, P], bf16, tag="ident")
        make_identity(nc, ident[:, :])

        # Software pipeline with 1-batch skew: segment b emits loads+scores+
        # exp for batch b, then PT+AO+store for batch b-1 (whose exp chain
        # finished during segment b's scores) so PE never stalls on the
        # stats->exp latency.
        stage = {}   # b -> (p_sb, inv, ve)
        loaded = {}  # b -> (q8, k8, dq8, ve)

        def emit_loads(b):
            q8 = in_p.tile([P, DB * N], fp8, tag="q8")
            k8 = in_p.tile([P, DB * N], fp8, tag="k8")
            srcs = [(k8, k8_d), (q8, q8_d)]
            dq8 = None
            if COMP:
                dq8 = in_p.tile([P, DB * N], fp8, tag="dq8")
                srcs.append((dq8, dq8_d))
            # split loads by chunk halves so the first score matmuls can
            # start before the whole tensor lands
            h = DB // 2
            views = []
            for sb, src in srcs:
                sbv = sb[:, :].rearrange("p (c n) -> p c n", c=DB)
                srcv = src[b].rearrange("(c p) n -> p c n", p=P)
                views.append((sbv, srcv))
            for lo, hi in ((0, h), (h, DB)):
                for sbv, srcv in views:
                    nc.sync.dma_start(sbv[:, lo:hi, :], srcv[:, lo:hi, :])
            ve = in_p.tile([P, NB * D], bf16, tag="ve")
            nc.sync.dma_start(
                ve[:, :].rearrange("p (t d) -> p t d", t=NB),
                ve_d[b].rearrange("(t p) d -> p t d", p=P))
            loaded[b] = (q8, k8, dq8, ve)

        def emit_front(b):
            q8, k8, dq8, ve = loaded.pop(b)
            q8v = q8[:, :].rearrange("p (c n) -> p c n", c=DB)
            k8v = k8[:, :].rearrange("p (c n) -> p c n", c=DB)
            groups = [q8v]
            if COMP:
                groups.append(dq8[:, :].rearrange("p (c n) -> p c n", c=DB))

            p_sb, inv = [], []
            for t in range(NB):
                s_ps = psS.tile([P, N], f32, tag="s")
                for cp in range(DP):
                    for g, qv in enumerate(groups):
                        nc.tensor.matmul(
                            s_ps[:, :],
                            qv[:, 2 * cp:2 * cp + 2, ts(t, P)],
                            k8v[:, 2 * cp:2 * cp + 2, :],
                            start=(cp == 0 and g == 0),
                            stop=(cp == DP - 1 and g == len(groups) - 1),
                            perf_mode=DR)
                st6 = st_p.tile([P, 6], f32, tag="st6")
                nc.vector.bn_stats(st6[:, :], s_ps[:, :])
                mv = st_p.tile([P, 2], f32, tag="mv")
                nc.vector.bn_aggr(mv[:, :], st6[:, :])
                # r = 1/sd = exp(-0.5*ln(var*N/(N-1)))
                lnv = st_p.tile([P, 1], f32, tag="lnv")
                nc.scalar.activation(lnv[:, :], mv[:, 1:2], AF.Ln,
                                     scale=float(N) / (N - 1.0))
                r = st_p.tile([P, 1], f32, tag="r")
                nc.scalar.activation(r[:, :], lnv[:, :], AF.Exp, scale=-0.5)
                pt_ = p_p.tile([P, N], bf16, tag="p")
                rs = st_p.tile([P, 1], f32, tag="rs")
                nc.scalar.activation(pt_[:, :], s_ps[:, :], AF.Exp,
                                     scale=r[:, :], accum_out=rs[:, :])
                iv = st_p.tile([P, 1], f32, tag="iv")
                nc.vector.reciprocal(iv[:, :], rs[:, :])
                p_sb.append(pt_)
                inv.append(iv)
            stage[b] = (p_sb, inv, ve)

        def emit_back(b):
            p_sb, inv, ve = stage.pop(b)
            ao_sb = ao_p.tile([P, NB * D], bf16, tag="ao")
            xts = []
            for t in range(NB):
                # transpose P_t -> [m, n-block-t], depends only on exp(t)
                tp = psA.tile([P, N], bf16, tag="tpose")
                for mc in range(NB):
                    nc.tensor.matmul(
                        tp[:, ts(mc, P)], p_sb[t][:, ts(mc, P)],
                        ident[:, :], is_transpose=True,
                        start=True, stop=True)
                xt = trp_p.tile([P, N], bf16, tag="ptr")
                if t % 2 == 0:
                    nc.scalar.copy(xt[:, :], tp[:, :])
                else:
                    nc.vector.tensor_copy(xt[:, :], tp[:, :])
                xts.append(xt)
            for t in range(NB):
                xt = xts[t]
                for dt_ in range(2):
                    a_ps = psO.tile([P, 512], f32, tag="aops")
                    for mc in range(NB):
                        nc.tensor.matmul(
                            a_ps[:, :], xt[:, ts(mc, P)],
                            ve[:, mc * D + dt_ * 512: mc * D + dt_ * 512 + 512],
                            start=(mc == 0), stop=(mc == NB - 1))
                    dst = ao_sb[:, t * D + dt_ * 512: t * D + dt_ * 512 + 512]
                    if dt_ == 0:
                        nc.scalar.mul(dst, a_ps[:, :], inv[t][:, :])
                    else:
                        nc.vector.tensor_scalar_mul(dst, a_ps[:, :],
                                                    inv[t][:, :])
                nc.sync.dma_start(
                    ao_d[b, t * P:(t + 1) * P, :],
                    ao_sb[:, ts(t, D)])

        emit_loads(0)
        for b in range(BPC + 1):
            if b < BPC:
                if b + 1 < BPC:
                    emit_loads(b + 1)
                emit_front(b)
            if b > 0:
                emit_back(b - 1)

    nc.compile()
    return nc


def kernel(**inputs):
    import ml_dtypes
    from concourse.bass_utils import run_bass_kernel_spmd

    global _last_in_maps
    f8 = ml_dtypes.float8_e4m3
    bf = ml_dtypes.bfloat16

    Q = np.asarray(inputs["Q"], dtype=np.float32)
    K = np.asarray(inputs["K"], dtype=np.float32)
    V = np.asarray(inputs["V"], dtype=np.float32)
    ids = np.asarray(inputs["electrode_ids"]).astype(np.int64)
    emb = np.asarray(inputs["emb"], dtype=np.float32)
    fc_w = np.asarray(inputs["fc_w"], dtype=np.float32)
    fc_b = np.asarray(inputs["fc_b"], dtype=np.float32)

    e = emb[ids]                                  # [N, D]
    QT = np.ascontiguousarray((Q + e).transpose(0, 2, 1))   # [B, D, N]
    KT = np.ascontiguousarray((K + e).transpose(0, 2, 1))
    Q8 = QT.astype(f8)
    K8 = KT.astype(f8)
    dQ8 = (QT - Q8.astype(np.float32)).astype(f8) if COMP else None
    Veb = np.ascontiguousarray((V + e).astype(bf))          # [B, N, D]

    if "prog" not in _prog_cache:
        _prog_cache["prog"] = _build_program()
    nc = _prog_cache["prog"]

    in_maps = []
    for i in range(NCORES):
        sl = slice(i * BPC, (i + 1) * BPC)
        m = {
            "q8t": np.ascontiguousarray(Q8[sl]),
            "k8t": np.ascontiguousarray(K8[sl]),
            "veb": Veb[sl],
        }
        if COMP:
            m["dq8t"] = np.ascontiguousarray(dQ8[sl])
        in_maps.append(m)
    _last_in_maps = in_maps
    res = run_bass_kernel_spmd(nc, in_maps, list(range(NCORES)))
    AO = np.concatenate([np.asarray(r["ao"]) for r in res.results], axis=0)
    AO = AO.astype(np.float32).reshape(B, N * D)
    out = AO @ fc_w.T + fc_b
    return np.ascontiguousarray(out.astype(np.float32))


# revision 20
# speedup vs baseline: 1.0408x; 1.0153x over previous
"""Trainium2 Bass kernel for NeuromodulatedAttentionWithElectrodes.

Math simplification (verified ~1e-6 vs the jax reference): the dopamine/
serotonin "mod" term is a per-row constant added to the score matrix;
row-std (any ddof) is shift-invariant and softmax is shift-invariant, so
that whole pipeline cancels. Per batch b:

    Qe, Ke, Ve = Q[b]+e, K[b]+e, V[b]+e        (e = emb[electrode_ids])
    Sraw = Qe @ Ke^T                           [N, N]
    r[n] = 1/rowstd(Sraw, ddof=1)
    P    = exp(Sraw * r[:, None]);  rs = rowsum(P)
    AO   = (P @ Ve) / rs[:, None]              [N, D]
    out_b = AO.flat @ fc_w.T + fc_b            [C]

Device computes Sraw/P/AO; the tiny fc contraction (0.025% of FLOPs) and
input prep (e-add, transpose, quantize) run on host.

Scores use fp8(e4m3) DoubleRow matmuls (K=256 per instruction); the AO
matmul stays bf16. End-to-end rel err ~1.56e-2 vs the 2e-2 budget; set
COMP=True for one-sided error compensation (Sraw ~= (Q8+dQ8) @ K8^T,
~1e-2) at ~+14% device time.

Sharding: data-parallel over batch, 8 batches per core on 8 cores.
"""

import numpy as np

B, N, D, C = 64, 512, 1024, 4
NCORES = 8
BPC = B // NCORES  # batches per core
P = 128            # partitions
NB = N // P        # 4 row blocks
DB = D // P        # 8 d chunks
DP = DB // 2       # 4 DoubleRow chunk-pairs

_prog_cache = {}
_last_in_maps = None
COMP = False  # one-sided fp8 error compensation for the score matmul


def _build_program():
    from contextlib import ExitStack

    import concourse.mybir as mybir
    import concourse.tile as tile
    from concourse import bacc
    from concourse.bass import ts
    from concourse.masks import make_identity

    f32 = mybir.dt.float32
    bf16 = mybir.dt.bfloat16
    fp8 = mybir.dt.float8e4
    AF = mybir.ActivationFunctionType
    ALU = mybir.AluOpType
    DR = mybir.MatmulPerfMode.DoubleRow

    # Keep Exp+Ln resident in one ACT table set (avoid reload ping-pong).
    from concourse import hw_specs as _hws
    if not getattr(bacc, "_act_tbl_patched", False):
        _orig_gat = _hws.get_activation_tables

        def _patched_gat(arch):
            t = _orig_gat(arch)
            AF_ = mybir.ActivationFunctionType
            for name, fns in t.items():
                if name != "natural_log_exp_and_others":
                    fns.discard(AF_.Exp)
                    fns.discard(AF_.Ln)
            return t

        bacc.get_activation_tables = _patched_gat
        bacc._act_tbl_patched = True

    nc = bacc.Bacc("TRN2", target_bir_lowering=False, debug=False,
                   num_devices=NCORES)
    q8_d = nc.declare_dram_parameter("q8t", [BPC, D, N], fp8, isOutput=False)
    dq8_d = (nc.declare_dram_parameter("dq8t", [BPC, D, N], fp8,
                                       isOutput=False) if COMP else None)
    k8_d = nc.declare_dram_parameter("k8t", [BPC, D, N], fp8, isOutput=False)
    ve_d = nc.declare_dram_parameter("veb", [BPC, N, D], bf16, isOutput=False)
    ao_d = nc.declare_dram_parameter("ao", [BPC, N, D], bf16, isOutput=True)

    with tile.TileContext(nc) as tc, ExitStack() as ctx:
        const_p = ctx.enter_context(tc.tile_pool(name="const", bufs=1))
        in_p = ctx.enter_context(tc.tile_pool(name="inp", bufs=3))
        p_p = ctx.enter_context(tc.tile_pool(name="p", bufs=8))
        trp_p = ctx.enter_context(tc.tile_pool(name="trp", bufs=8))
        ao_p = ctx.enter_context(tc.tile_pool(name="ao", bufs=3))
        st_p = ctx.enter_context(tc.tile_pool(name="st", bufs=24))
        psS = ctx.enter_context(tc.tile_pool(name="psS", bufs=2, space="PSUM"))
        psA = ctx.enter_context(tc.tile_pool(name="psA", bufs=4, space="PSUM"))
        psO = ctx.enter_context(tc.tile_pool(name="psO", bufs=2, space="PSUM"))

        ident = const_p.tile([P, P], bf16, tag="ident")
        make_identity(nc, ident[:, :])
        identf = const_p.tile([P, P], f32, tag="identf")
        nc.scalar.copy(identf[:, :], ident[:, :])
        warm = psS.tile([P, N], f32, tag="s")
        for _ in range(10):
            nc.tensor.matmul(warm[:, 0:P], identf[:, :], identf[:, :],
                             is_transpose=True, start=True, stop=True)

        # Software pipeline with 1-batch skew: segment b emits loads+scores+
        # exp for batch b, then PT+AO+store for batch b-1 (whose exp chain
        # finished during segment b's scores) so PE never stalls on the
        # stats->exp latency.
        stage = {}   # b -> (p_sb, inv, ve)
        loaded = {}  # b -> (q8, k8, dq8, ve)

        def emit_loads(b):
            q8 = in_p.tile([P, DB * N], fp8, tag="q8")
            k8 = in_p.tile([P, DB * N], fp8, tag="k8")
            srcs = [(k8, k8_d), (q8, q8_d)]
            dq8 = None
            if COMP:
                dq8 = in_p.tile([P, DB * N], fp8, tag="dq8")
                srcs.append((dq8, dq8_d))
            # split loads by chunk halves so the first score matmuls can
            # start before the whole tensor lands
            h = DB // 2
            views = []
            for sb, src in srcs:
                sbv = sb[:, :].rearrange("p (c n) -> p c n", c=DB)
                srcv = src[b].rearrange("(c p) n -> p c n", p=P)
                views.append((sbv, srcv))
            for lo, hi in ((0, h), (h, DB)):
                for sbv, srcv in views:
                    nc.sync.dma_start(sbv[:, lo:hi, :], srcv[:, lo:hi, :])
            ve = in_p.tile([P, NB * D], bf16, tag="ve")
            nc.sync.dma_start(
                ve[:, :].rearrange("p (t d) -> p t d", t=NB),
                ve_d[b].rearrange("(t p) d -> p t d", p=P))
            loaded[b] = (q8, k8, dq8, ve)

        def emit_front(b):
            q8, k8, dq8, ve = loaded.pop(b)
            q8v = q8[:, :].rearrange("p (c n) -> p c n", c=DB)
            k8v = k8[:, :].rearrange("p (c n) -> p c n", c=DB)
            groups = [q8v]
            if COMP:
                groups.append(dq8[:, :].rearrange("p (c n) -> p c n", c=DB))

            p_sb, inv = [], []
            for t in range(NB):
                s_ps = psS.tile([P, N], f32, tag="s")
                for cp in range(DP):
                    for g, qv in enumerate(groups):
                        nc.tensor.matmul(
                            s_ps[:, :],
                            qv[:, 2 * cp:2 * cp + 2, ts(t, P)],
                            k8v[:, 2 * cp:2 * cp + 2, :],
                            start=(cp == 0 and g == 0),
                            stop=(cp == DP - 1 and g == len(groups) - 1),
                            perf_mode=DR)
                st6 = st_p.tile([P, 6], f32, tag="st6")
                nc.vector.bn_stats(st6[:, :], s_ps[:, :])
                mv = st_p.tile([P, 2], f32, tag="mv")
                nc.vector.bn_aggr(mv[:, :], st6[:, :])
                # r = 1/sd = exp(-0.5*ln(var*N/(N-1)))
                lnv = st_p.tile([P, 1], f32, tag="lnv")
                nc.scalar.activation(lnv[:, :], mv[:, 1:2], AF.Ln,
                                     scale=float(N) / (N - 1.0))
                r = st_p.tile([P, 1], f32, tag="r")
                nc.scalar.activation(r[:, :], lnv[:, :], AF.Exp, scale=-0.5)
                pt_ = p_p.tile([P, N], bf16, tag="p")
                rs = st_p.tile([P, 1], f32, tag="rs")
                nc.scalar.activation(pt_[:, :], s_ps[:, :], AF.Exp,
                                     scale=r[:, :], accum_out=rs[:, :])
                iv = st_p.tile([P, 1], f32, tag="iv")
                nc.vector.reciprocal(iv[:, :], rs[:, :])
                p_sb.append(pt_)
                inv.append(iv)
            stage[b] = (p_sb, inv, ve)

        def emit_back(b):
            p_sb, inv, ve = stage.pop(b)
            ao_sb = ao_p.tile([P, NB * D], bf16, tag="ao")
            xts = []
            for t in range(NB):
                # transpose P_t -> [m, n-block-t], depends only on exp(t)
                tp = psA.tile([P, N], bf16, tag="tpose")
                for mc in range(NB):
                    nc.tensor.matmul(
                        tp[:, ts(mc, P)], p_sb[t][:, ts(mc, P)],
                        ident[:, :], is_transpose=True,
                        start=True, stop=True)
                xt = trp_p.tile([P, N], bf16, tag="ptr")
                if t % 2 == 0:
                    nc.scalar.copy(xt[:, :], tp[:, :])
                else:
                    nc.vector.tensor_copy(xt[:, :], tp[:, :])
                xts.append(xt)
            for t in range(NB):
                xt = xts[t]
                for dt_ in range(2):
                    a_ps = psO.tile([P, 512], f32, tag="aops")
                    for mc in range(NB):
                        nc.tensor.matmul(
                            a_ps[:, :], xt[:, ts(mc, P)],
                            ve[:, mc * D + dt_ * 512: mc * D + dt_ * 512 + 512],
                            start=(mc == 0), stop=(mc == NB - 1))
                    dst = ao_sb[:, t * D + dt_ * 512: t * D + dt_ * 512 + 512]
                    if dt_ == 0:
                        nc.scalar.mul(dst, a_ps[:, :], inv[t][:, :])
                    else:
                        nc.vector.tensor_scalar_mul(dst, a_ps[:, :],
                                                    inv[t][:, :])
                    nc.sync.dma_start(
                        ao_d[b, t * P:(t + 1) * P, dt_ * 512:dt_ * 512 + 512],
                        ao_sb[:, t * D + dt_ * 512: t * D + dt_ * 512 + 512])

        emit_loads(0)
        for b in range(BPC + 1):
            if b < BPC:
                if b + 1 < BPC:
                    emit_loads(b + 1)
                emit_front(b)
            if b > 0:
                emit_back(b - 1)

    nc.compile()
    return nc


def kernel(**inputs):
    import ml_dtypes
    from concourse.bass_utils import run_bass_kernel_spmd

    global _last_in_maps
    f8 = ml_dtypes.float8_e4m3
    bf = ml_dtypes.bfloat16

    Q = np.asarray(inputs["Q"], dtype=np.float32)
    K = np.asarray(inputs["K"], dtype=np.float32)
    V = np.asarray(inputs["V"], dtype=np.float32)
    ids = np.asarray(inputs["electrode_ids"]).astype(np.int64)
    emb = np.asarray(inputs["emb"], dtype=np.float32)
    fc_w = np.asarray(inputs["fc_w"], dtype=np.float32)
    fc_b = np.asarray(inputs["fc_b"], dtype=np.float32)

    e = emb[ids]                                  # [N, D]
    QT = np.ascontiguousarray((Q + e).transpose(0, 2, 1))   # [B, D, N]
    KT = np.ascontiguousarray((K + e).transpose(0, 2, 1))
    Q8 = QT.astype(f8)
    K8 = KT.astype(f8)
    dQ8 = (QT - Q8.astype(np.float32)).astype(f8) if COMP else None
    Veb = np.ascontiguousarray((V + e).astype(bf))          # [B, N, D]

    if "prog" not in _prog_cache:
        _prog_cache["prog"] = _build_program()
    nc = _prog_cache["prog"]

    in_maps = []
    for i in range(NCORES):
        sl = slice(i * BPC, (i + 1) * BPC)
        m = {
            "q8t": np.ascontiguousarray(Q8[sl]),
            "k8t": np.ascontiguousarray(K8[sl]),
            "veb": Veb[sl],
        }
        if COMP:
            m["dq8t"] = np.ascontiguousarray(dQ8[sl])
        in_maps.append(m)
    _last_in_maps = in_maps
    res = run_bass_kernel_spmd(nc, in_maps, list(range(NCORES)))
    AO = np.concatenate([np.asarray(r["ao"]) for r in res.results], axis=0)
    AO = AO.astype(np.float32).reshape(B, N * D)
    out = AO @ fc_w.T + fc_b
    return np.ascontiguousarray(out.astype(np.float32))
